# revision 6
# baseline (speedup 1.0000x reference)
"""Complex attention kernel for 8 TRN2 NeuronCores (SPMD).

Sharding: core c -> batch b=c//2, head-group hg=c%2 (8 of 16 heads).
Stage 1 computes q/k (transposed layout, complex parts packed along
partitions with sign folded into host-packed weights), v in natural
layout. Attention works on s^T[k,q] blocks so no on-chip transposes are
needed; softmax runs without max-subtraction (logits = |s|/8 >= 0).
A pairwise AllGather exchanges attention outputs before the output
projection; each core computes half the projection columns.

All matmuls in float32r (full PE rate at N>=256, ~1e-4 matmul error).
"""
from contextlib import ExitStack

import numpy as np

import concourse.bass as bass
import concourse.tile as tile
from concourse import bacc, mybir
from concourse.bass_utils import run_bass_kernel_spmd

B, S, D, H = 4, 1024, 1024, 16
HD = 64          # head dim
HPC = 8          # heads per core
N_CORES = 8
NEG = -300.0     # mask bias: exp(u + NEG) == 0 in fp32

F32 = mybir.dt.float32
F32R = mybir.dt.float32r

_CACHE = {}


def _build():
    nc = bacc.Bacc("TRN2", target_bir_lowering=False, debug=False, num_devices=N_CORES)

    # ---- I/O ----
    xrT = nc.dram_tensor("xrT", [D, S], F32R, kind="ExternalInput").ap()
    xiT = nc.dram_tensor("xiT", [D, S], F32R, kind="ExternalInput").ap()
    WA = nc.dram_tensor("WA", [D, 4 * 1024], F32R, kind="ExternalInput").ap()
    WB = nc.dram_tensor("WB", [D, 4 * 1024], F32R, kind="ExternalInput").ap()
    Wpr = nc.dram_tensor("Wpr", [2048, 512], F32R, kind="ExternalInput").ap()
    Wpi = nc.dram_tensor("Wpi", [2048, 512], F32R, kind="ExternalInput").ap()
    maskf = nc.dram_tensor("maskf", [128, 640], F32, kind="ExternalInput").ap()
    y = nc.dram_tensor("y", [2, S, 512], F32, kind="ExternalOutput").ap()

    # ---- internal DRAM ----
    qriT_d = nc.dram_tensor("qriT_d", [1024, S], F32R)
    q2T_d = nc.dram_tensor("q2T_d", [1024, S], F32R)
    kriT_d = nc.dram_tensor("kriT_d", [1024, S], F32R)
    v_d = nc.dram_tensor("v_d", [S, 1024], F32R)
    otA = nc.dram_tensor("otA", [512, S], F32R)   # heads 0-3 outT
    otB = nc.dram_tensor("otB", [512, S], F32R)   # heads 4-7 outT
    gA = nc.dram_tensor("gA", [1024, S], F32R)    # gathered heads {0-3, 8-11}
    gB = nc.dram_tensor("gB", [1024, S], F32R)    # gathered heads {4-7, 12-15}

    groups = [[0, 1], [2, 3], [4, 5], [6, 7]]

    with tile.TileContext(nc) as tc:
        with ExitStack() as ctx:
            singles = ctx.enter_context(tc.tile_pool(name="singles", bufs=1))
            mask_sb = singles.tile([128, 640], F32)
            nc.sync.dma_start(out=mask_sb, in_=maskf)
            ones32 = singles.tile([128, 128], F32)
            nc.vector.memset(ones32, 1.0)
            ones_sb = singles.tile([128, 128], F32R)
            nc.vector.tensor_copy(ones_sb, ones32)
            nln8 = singles.tile([128, 1], F32)
            nc.vector.memset(nln8, -0.5 * np.log(float(HD)))  # exp(0.5 ln t + this) = sqrt(t)/sqrt(HD)

            # ================= stage 1: qkv projections =================
            with ExitStack() as p1:
                xpool = p1.enter_context(tc.tile_pool(name="xpool", bufs=1))
                wpool = p1.enter_context(tc.tile_pool(name="wpool", bufs=2))
                spool = p1.enter_context(tc.tile_pool(name="spool", bufs=3))
                ps1 = p1.enter_context(tc.tile_pool(name="ps1", bufs=2, space="PSUM"))
                psv = p1.enter_context(tc.tile_pool(name="psv", bufs=4, space="PSUM"))

                xr_sb = xpool.tile([128, 8, S], F32R)
                xi_sb = xpool.tile([128, 8, S], F32R)
                nc.sync.dma_start(out=xr_sb, in_=xrT.rearrange("(dc p) s -> p dc s", p=128))
                nc.sync.dma_start(out=xi_sb, in_=xiT.rearrange("(dc p) s -> p dc s", p=128))

                # v first (natural layout), so per-head attention can start early
                for cc in range(2):
                    wv = wpool.tile([128, 8, 512], F32R, tag="wv")
                    wvb = wpool.tile([128, 8, 512], F32R, tag="wvb")
                    cb = 3072 + cc * 512
                    nc.sync.dma_start(out=wv, in_=WA[:, cb:cb + 512].rearrange("(dc p) f -> p dc f", p=128))
                    nc.sync.dma_start(out=wvb, in_=WB[:, cb:cb + 512].rearrange("(dc p) f -> p dc f", p=128))
                    for m in range(8):
                        ps = psv.tile([128, 512], F32, tag="v")
                        ms = slice(m * 128, (m + 1) * 128)
                        for dc in range(8):
                            nc.tensor.matmul(ps, xr_sb[:, dc, ms], wv[:, dc, :],
                                             start=(dc == 0), stop=False)
                            nc.tensor.matmul(ps, xi_sb[:, dc, ms], wvb[:, dc, :],
                                             start=False, stop=(dc == 7))
                        st = spool.tile([128, 512], F32R, tag="stv")
                        nc.scalar.copy(st, ps)
                        nc.sync.dma_start(out=v_d.ap()[ms, cc * 512:(cc + 1) * 512], in_=st)

                # q/q2/k transposed layout, grouped per head
                for h in range(HPC):
                    for cb, dest in ((h * 128, qriT_d), (1024 + h * 128, q2T_d),
                                     (2048 + h * 128, kriT_d)):
                        wa = wpool.tile([128, 8, 128], F32R, tag="wa")
                        wb = wpool.tile([128, 8, 128], F32R, tag="wb")
                        nc.sync.dma_start(out=wa, in_=WA[:, cb:cb + 128].rearrange("(dc p) f -> p dc f", p=128))
                        nc.sync.dma_start(out=wb, in_=WB[:, cb:cb + 128].rearrange("(dc p) f -> p dc f", p=128))
                        ps0 = ps1.tile([128, 512], F32, tag="t0")
                        psx = ps1.tile([128, 512], F32, tag="t1")
                        for dc in range(8):
                            nc.tensor.matmul(ps0, wa[:, dc, :], xr_sb[:, dc, 0:512],
                                             start=(dc == 0), stop=False)
                            nc.tensor.matmul(psx, wa[:, dc, :], xr_sb[:, dc, 512:1024],
                                             start=(dc == 0), stop=False)
                            nc.tensor.matmul(ps0, wb[:, dc, :], xi_sb[:, dc, 0:512],
                                             start=False, stop=(dc == 7))
                            nc.tensor.matmul(psx, wb[:, dc, :], xi_sb[:, dc, 512:1024],
                                             start=False, stop=(dc == 7))
                        st = spool.tile([128, 1024], F32R, tag="st")
                        nc.scalar.copy(st[:, 0:512], ps0)
                        nc.scalar.copy(st[:, 512:1024], psx)
                        hs = slice(h * 128, (h + 1) * 128)
                        nc.sync.dma_start(out=dest.ap()[hs, :], in_=st)

            # ================= attention per head =================
            with ExitStack() as p2:
                apool = p2.enter_context(tc.tile_pool(name="apool", bufs=2))
                epool = p2.enter_context(tc.tile_pool(name="epool", bufs=2))
                opool = p2.enter_context(tc.tile_pool(name="opool", bufs=3))
                psS = p2.enter_context(tc.tile_pool(name="psS", bufs=2, space="PSUM"))
                psA = p2.enter_context(tc.tile_pool(name="psA", bufs=2, space="PSUM"))

                for h in range(HPC):
                    hs = slice(h * 128, (h + 1) * 128)
                    qri = apool.tile([128, S], F32R, tag="qri")
                    q2 = apool.tile([128, S], F32R, tag="q2")
                    kri = apool.tile([128, S], F32R, tag="kri")
                    vsb = apool.tile([128, 8, 128], F32R, tag="v")
                    nc.sync.dma_start(out=qri, in_=qriT_d.ap()[hs, :])
                    nc.sync.dma_start(out=q2, in_=q2T_d.ap()[hs, :])
                    nc.sync.dma_start(out=kri, in_=kriT_d.ap()[hs, :])
                    nc.sync.dma_start(out=vsb, in_=v_d.ap()[:, hs].rearrange("(kt p) f -> p kt f", p=128))

                    for qc in range(2):
                        qs = slice(qc * 512, (qc + 1) * 512)
                        avp = psA.tile([128, 512], F32, tag="av")
                        rbp = psA.tile([128, 512], F32, tag="rb")
                        nkt = 4 * (qc + 1)
                        for kt in range(nkt):
                            lhsT = kri[:, kt * 128:(kt + 1) * 128]
                            sre = psS.tile([128, 512], F32, tag="sre")
                            sim = psS.tile([128, 512], F32, tag="sim")
                            nc.tensor.matmul(sre, lhsT, qri[:, qs], start=True, stop=True)
                            nc.tensor.matmul(sim, lhsT, q2[:, qs], start=True, stop=True)
                            c1 = epool.tile([128, 512], F32, tag="c1")
                            nc.vector.tensor_copy(c1, sre)
                            t = epool.tile([128, 512], F32, tag="t")
                            nc.vector.tensor_mul(t, c1, sre)
                            c2 = epool.tile([128, 512], F32, tag="c2")
                            nc.vector.tensor_copy(c2, sim)
                            t2 = epool.tile([128, 512], F32, tag="t2")
                            nc.vector.tensor_mul(t2, c2, sim)
                            u = epool.tile([128, 512], F32, tag="u")
                            nc.gpsimd.tensor_add(u, t, t2)
                            # logit = sqrt(u)/sqrt(HD) = exp(0.5 ln u - 0.5 ln HD);
                            # Ln and Exp share one ACT table set (no reloads)
                            w1 = epool.tile([128, 512], F32, tag="w1")
                            nc.scalar.activation(w1, u, mybir.ActivationFunctionType.Ln)
                            u2 = epool.tile([128, 512], F32, tag="u2")
                            nc.scalar.activation(u2, w1, mybir.ActivationFunctionType.Exp,
                                                 bias=nln8, scale=0.5)
                            o = kt * 128 - qc * 512
                            if o >= 0:  # diagonal-crossing block: mask cols < k
                                nc.vector.tensor_add(u2[:, 0:o + 128], u2[:, 0:o + 128],
                                                     mask_sb[:, 512 - o:640])
                            e = epool.tile([128, 512], F32R, tag="e")
                            nc.scalar.activation(e, u2, mybir.ActivationFunctionType.Exp)
                            nc.tensor.matmul(avp, vsb[:, kt, :], e,
                                             start=(kt == 0), stop=(kt == nkt - 1))
                            nc.tensor.matmul(rbp, ones_sb, e,
                                             start=(kt == 0), stop=(kt == nkt - 1))
                        rr = epool.tile([128, 512], F32, tag="rr")
                        nc.vector.reciprocal(rr, rbp)
                        ot = opool.tile([128, 512], F32R, tag="ot")
                        nc.vector.tensor_mul(ot, avp, rr)
                        dst = otA if h < 4 else otB
                        hh = h if h < 4 else h - 4
                        nc.sync.dma_start(out=dst.ap()[hh * 128:(hh + 1) * 128, qs], in_=ot)

                    if h == 3:
                        nc.gpsimd.collective_compute(
                            "AllGather", mybir.AluOpType.bypass,
                            ins=[otA.ap()], outs=[gA.ap()], replica_groups=groups)
                if True:
                    nc.gpsimd.collective_compute(
                        "AllGather", mybir.AluOpType.bypass,
                        ins=[otB.ap()], outs=[gB.ap()], replica_groups=groups)

            # ================= output projection =================
            with ExitStack() as p3:
                ppool = p3.enter_context(tc.tile_pool(name="ppool", bufs=1))
                lpool = p3.enter_context(tc.tile_pool(name="lpool", bufs=2))
                ypool = p3.enter_context(tc.tile_pool(name="ypool", bufs=3))
                psP = p3.enter_context(tc.tile_pool(name="psP", bufs=2, space="PSUM"))

                wpr_sb = ppool.tile([128, 16, 512], F32R)
                wpi_sb = ppool.tile([128, 16, 512], F32R)
                nc.sync.dma_start(out=wpr_sb, in_=Wpr.rearrange("(fc p) c -> p fc c", p=128))
                nc.sync.dma_start(out=wpi_sb, in_=Wpi.rearrange("(fc p) c -> p fc c", p=128))

                for m in range(8):
                    ms = slice(m * 128, (m + 1) * 128)
                    lha = lpool.tile([128, 8, 128], F32R, tag="lha")
                    lhb = lpool.tile([128, 8, 128], F32R, tag="lhb")
                    nc.sync.dma_start(out=lha, in_=gA.ap()[:, ms].rearrange("(fc p) s -> p fc s", p=128))
                    nc.sync.dma_start(out=lhb, in_=gB.ap()[:, ms].rearrange("(fc p) s -> p fc s", p=128))
                    pyr = psP.tile([128, 512], F32, tag="yr")
                    pyi = psP.tile([128, 512], F32, tag="yi")
                    for fc in range(16):
                        lh = lha[:, fc, :] if fc < 8 else lhb[:, fc - 8, :]
                        nc.tensor.matmul(pyr, lh, wpr_sb[:, fc, :],
                                         start=(fc == 0), stop=(fc == 15))
                        nc.tensor.matmul(pyi, lh, wpi_sb[:, fc, :],
                                         start=(fc == 0), stop=(fc == 15))
                    syr = ypool.tile([128, 512], F32, tag="syr")
                    nc.scalar.copy(syr, pyr)
                    nc.sync.dma_start(out=y[0, ms, :], in_=syr)
                    syi = ypool.tile([128, 512], F32, tag="syi")
                    nc.vector.tensor_copy(syi, pyi)
                    nc.sync.dma_start(out=y[1, ms, :], in_=syi)

    nc.compile()
    return nc


def _prep_inputs(x_re, x_im, wqkv_re, wqkv_im, wproj_re, wproj_im):
    """Pack per-core input maps (all host-side numpy)."""
    f32 = np.float32

    def qkv_rows(kind, g):
        off = {"q": 0, "k": 1024, "v": 2048}[kind]
        return slice(off + g * 64, off + (g + 1) * 64)

    # mask: cols 0-511 = NEG; cols 512+jj: 0 if jj >= k else NEG
    maskf = np.full((128, 640), NEG, f32)
    k_idx = np.arange(128)[:, None]
    jj = np.arange(128)[None, :]
    maskf[:, 512:] = np.where(jj >= k_idx, 0.0, NEG)

    WA_hg, WB_hg, Wpr_hg, Wpi_hg = {}, {}, {}, {}
    fc_order = [0, 1, 2, 3, 8, 9, 10, 11, 4, 5, 6, 7, 12, 13, 14, 15]
    for hg in range(2):
        WA = np.empty((D, 4096), f32)
        WB = np.empty((D, 4096), f32)
        for h in range(HPC):
            g = hg * HPC + h
            Wqr = wqkv_re[qkv_rows("q", g)]   # [64, D]
            Wqi = wqkv_im[qkv_rows("q", g)]
            Wkr = wqkv_re[qkv_rows("k", g)]
            Wki = wqkv_im[qkv_rows("k", g)]
            Wvr = wqkv_re[qkv_rows("v", g)]
            Wvi = wqkv_im[qkv_rows("v", g)]
            c = h * 128
            # qriT = [qr; qi]
            WA[:, c:c + 64] = Wqr.T;        WA[:, c + 64:c + 128] = Wqi.T
            WB[:, c:c + 64] = -Wqi.T;       WB[:, c + 64:c + 128] = Wqr.T
            # q2T = [qi; -qr]
            c = 1024 + h * 128
            WA[:, c:c + 64] = Wqi.T;        WA[:, c + 64:c + 128] = -Wqr.T
            WB[:, c:c + 64] = Wqr.T;        WB[:, c + 64:c + 128] = Wqi.T
            # kriT = [kr; ki]
            c = 2048 + h * 128
            WA[:, c:c + 64] = Wkr.T;        WA[:, c + 64:c + 128] = Wki.T
            WB[:, c:c + 64] = -Wki.T;       WB[:, c + 64:c + 128] = Wkr.T
            # v natural = [vr; vi]
            c = 3072 + h * 128
            WA[:, c:c + 64] = Wvr.T;        WA[:, c + 64:c + 128] = Wvi.T
            WB[:, c:c + 64] = -Wvi.T;       WB[:, c + 64:c + 128] = Wvr.T
        WA_hg[hg], WB_hg[hg] = np.ascontiguousarray(WA), np.ascontiguousarray(WB)

        cols = slice(hg * 512, (hg + 1) * 512)
        Wpr = np.empty((2048, 512), f32)
        Wpi = np.empty((2048, 512), f32)
        for fci, g in enumerate(fc_order):
            gs = slice(g * 64, (g + 1) * 64)
            r = fci * 128
            Wpr[r:r + 64] = wproj_re[cols, gs].T
            Wpr[r + 64:r + 128] = -wproj_im[cols, gs].T
            Wpi[r:r + 64] = wproj_im[cols, gs].T
            Wpi[r + 64:r + 128] = wproj_re[cols, gs].T
        Wpr_hg[hg], Wpi_hg[hg] = np.ascontiguousarray(Wpr), np.ascontiguousarray(Wpi)

    xT = {}
    for b in range(B):
        xT[b] = (np.ascontiguousarray(x_re[b].T.astype(f32)),
                 np.ascontiguousarray(x_im[b].T.astype(f32)))

    in_maps = []
    for c in range(N_CORES):
        b, hg = c // 2, c % 2
        in_maps.append({
            "xrT": xT[b][0], "xiT": xT[b][1],
            "WA": WA_hg[hg], "WB": WB_hg[hg],
            "Wpr": Wpr_hg[hg], "Wpi": Wpi_hg[hg],
            "maskf": maskf,
        })
    return in_maps


def _get_nc():
    if "nc" not in _CACHE:
        _CACHE["nc"] = _build()
    return _CACHE["nc"]


def kernel(x_re, x_im, wqkv_re, wqkv_im, wproj_re, wproj_im, _trace=False):
    nc = _get_nc()
    in_maps = _prep_inputs(np.asarray(x_re, np.float32), np.asarray(x_im, np.float32),
                           np.asarray(wqkv_re, np.float32), np.asarray(wqkv_im, np.float32),
                           np.asarray(wproj_re, np.float32), np.asarray(wproj_im, np.float32))
    res = run_bass_kernel_spmd(nc, in_maps, list(range(N_CORES)), trace=_trace)
    out = np.empty((2, B, S, D), np.float32)
    for c in range(N_CORES):
        b, hg = c // 2, c % 2
        yc = res.results[c]["y"]
        out[0, b, :, hg * 512:(hg + 1) * 512] = yc[0]
        out[1, b, :, hg * 512:(hg + 1) * 512] = yc[1]
    if _trace:
        return out, res
    return out


# revision 7
# speedup vs baseline: 1.1990x; 1.1990x over previous
"""Complex attention kernel for 8 TRN2 NeuronCores (SPMD).

Sharding: core c -> batch b=c//2, head-group hg=c%2 (8 of 16 heads).
Stage 1 computes q/k (transposed layout, complex parts packed along
partitions with sign folded into host-packed weights), v in natural
layout. Attention works on s^T[k,q] blocks so no on-chip transposes are
needed; softmax runs without max-subtraction (logits = |s|/8 >= 0).
A pairwise AllGather exchanges attention outputs before the output
projection; each core computes half the projection columns.

All matmuls in float32r (full PE rate at N>=256, ~1e-4 matmul error).
"""
from contextlib import ExitStack

import numpy as np

import concourse.bass as bass
import concourse.tile as tile
from concourse import bacc, mybir
from concourse.bass_utils import run_bass_kernel_spmd

B, S, D, H = 4, 1024, 1024, 16
HD = 64          # head dim
HPC = 8          # heads per core
N_CORES = 8
NEG = -300.0     # mask bias: exp(u + NEG) == 0 in fp32

F32 = mybir.dt.float32
F32R = mybir.dt.float32r

_CACHE = {}


def _patch_act_tables():
    """Make natural_log_exp_and_others the only set containing Ln/Exp so the
    act-table-load pass keeps one table set resident through the attention
    phase (instead of ping-ponging exp_and_others <-> natural_log, ~2.7us
    per reload). Only set *contents* are filtered; set order/indices are
    unchanged, so act_func_set_id stays valid for walrus."""
    if _CACHE.get("act_patched"):
        return
    import concourse.bacc as _bacc
    import concourse.hw_specs as _hw
    orig = _hw.get_activation_tables

    def patched(arch):
        tabs = dict(orig(arch))
        out = {}
        for name, fns in tabs.items():
            if name != "natural_log_exp_and_others":
                fns = {f for f in fns
                       if f not in (mybir.ActivationFunctionType.Exp,
                                    mybir.ActivationFunctionType.Ln)}
            out[name] = fns
        return out

    _bacc.get_activation_tables = patched
    _CACHE["act_patched"] = True


def _build():
    _patch_act_tables()
    nc = bacc.Bacc("TRN2", target_bir_lowering=False, debug=False, num_devices=N_CORES)

    # ---- I/O ----
    xrT = nc.dram_tensor("xrT", [D, S], F32R, kind="ExternalInput").ap()
    xiT = nc.dram_tensor("xiT", [D, S], F32R, kind="ExternalInput").ap()
    WA = nc.dram_tensor("WA", [D, 4 * 1024], F32R, kind="ExternalInput").ap()
    WB = nc.dram_tensor("WB", [D, 4 * 1024], F32R, kind="ExternalInput").ap()
    Wpr = nc.dram_tensor("Wpr", [2048, 512], F32R, kind="ExternalInput").ap()
    Wpi = nc.dram_tensor("Wpi", [2048, 512], F32R, kind="ExternalInput").ap()
    maskf = nc.dram_tensor("maskf", [128, 640], F32, kind="ExternalInput").ap()
    y = nc.dram_tensor("y", [2, S, 512], F32, kind="ExternalOutput").ap()

    # ---- internal DRAM ----
    qriT_d = nc.dram_tensor("qriT_d", [1024, S], F32R)
    q2T_d = nc.dram_tensor("q2T_d", [1024, S], F32R)
    kriT_d = nc.dram_tensor("kriT_d", [1024, S], F32R)
    v_d = nc.dram_tensor("v_d", [S, 1024], F32R)
    otA = nc.dram_tensor("otA", [512, S], F32R)   # heads 0-3 outT
    otB = nc.dram_tensor("otB", [512, S], F32R)   # heads 4-7 outT
    gA = nc.dram_tensor("gA", [1024, S], F32R)    # gathered heads {0-3, 8-11}
    gB = nc.dram_tensor("gB", [1024, S], F32R)    # gathered heads {4-7, 12-15}

    groups = [[0, 1], [2, 3], [4, 5], [6, 7]]

    with tile.TileContext(nc) as tc:
        with ExitStack() as ctx:
            singles = ctx.enter_context(tc.tile_pool(name="singles", bufs=1))
            mask_sb = singles.tile([128, 640], F32)
            nc.sync.dma_start(out=mask_sb, in_=maskf)
            ones32 = singles.tile([128, 128], F32)
            nc.vector.memset(ones32, 1.0)
            ones_sb = singles.tile([128, 128], F32R)
            nc.vector.tensor_copy(ones_sb, ones32)
            nln8 = singles.tile([128, 1], F32)
            nc.vector.memset(nln8, -0.5 * np.log(float(HD)))  # exp(0.5 ln t + this) = sqrt(t)/sqrt(HD)

            # ================= stage 1: qkv projections =================
            with ExitStack() as p1:
                xpool = p1.enter_context(tc.tile_pool(name="xpool", bufs=1))
                wpool = p1.enter_context(tc.tile_pool(name="wpool", bufs=2))
                spool = p1.enter_context(tc.tile_pool(name="spool", bufs=3))
                ps1 = p1.enter_context(tc.tile_pool(name="ps1", bufs=2, space="PSUM"))
                psv = p1.enter_context(tc.tile_pool(name="psv", bufs=4, space="PSUM"))

                xr_sb = xpool.tile([128, 8, S], F32R)
                xi_sb = xpool.tile([128, 8, S], F32R)
                nc.sync.dma_start(out=xr_sb, in_=xrT.rearrange("(dc p) s -> p dc s", p=128))
                nc.sync.dma_start(out=xi_sb, in_=xiT.rearrange("(dc p) s -> p dc s", p=128))

                # v first (natural layout), so per-head attention can start early
                for cc in range(2):
                    wv = wpool.tile([128, 8, 512], F32R, tag="wv")
                    wvb = wpool.tile([128, 8, 512], F32R, tag="wvb")
                    cb = 3072 + cc * 512
                    nc.sync.dma_start(out=wv, in_=WA[:, cb:cb + 512].rearrange("(dc p) f -> p dc f", p=128))
                    nc.sync.dma_start(out=wvb, in_=WB[:, cb:cb + 512].rearrange("(dc p) f -> p dc f", p=128))
                    for m in range(8):
                        ps = psv.tile([128, 512], F32, tag="v")
                        ms = slice(m * 128, (m + 1) * 128)
                        for dc in range(8):
                            nc.tensor.matmul(ps, xr_sb[:, dc, ms], wv[:, dc, :],
                                             start=(dc == 0), stop=False)
                            nc.tensor.matmul(ps, xi_sb[:, dc, ms], wvb[:, dc, :],
                                             start=False, stop=(dc == 7))
                        st = spool.tile([128, 512], F32R, tag="stv")
                        nc.scalar.copy(st, ps)
                        nc.sync.dma_start(out=v_d.ap()[ms, cc * 512:(cc + 1) * 512], in_=st)

                # q/q2/k transposed layout, grouped per head
                for h in range(HPC):
                    for cb, dest in ((h * 128, qriT_d), (1024 + h * 128, q2T_d),
                                     (2048 + h * 128, kriT_d)):
                        wa = wpool.tile([128, 8, 128], F32R, tag="wa")
                        wb = wpool.tile([128, 8, 128], F32R, tag="wb")
                        nc.sync.dma_start(out=wa, in_=WA[:, cb:cb + 128].rearrange("(dc p) f -> p dc f", p=128))
                        nc.sync.dma_start(out=wb, in_=WB[:, cb:cb + 128].rearrange("(dc p) f -> p dc f", p=128))
                        ps0 = ps1.tile([128, 512], F32, tag="t0")
                        psx = ps1.tile([128, 512], F32, tag="t1")
                        for dc in range(8):
                            nc.tensor.matmul(ps0, wa[:, dc, :], xr_sb[:, dc, 0:512],
                                             start=(dc == 0), stop=False)
                            nc.tensor.matmul(psx, wa[:, dc, :], xr_sb[:, dc, 512:1024],
                                             start=(dc == 0), stop=False)
                            nc.tensor.matmul(ps0, wb[:, dc, :], xi_sb[:, dc, 0:512],
                                             start=False, stop=(dc == 7))
                            nc.tensor.matmul(psx, wb[:, dc, :], xi_sb[:, dc, 512:1024],
                                             start=False, stop=(dc == 7))
                        st = spool.tile([128, 1024], F32R, tag="st")
                        nc.scalar.copy(st[:, 0:512], ps0)
                        nc.scalar.copy(st[:, 512:1024], psx)
                        hs = slice(h * 128, (h + 1) * 128)
                        nc.sync.dma_start(out=dest.ap()[hs, :], in_=st)

            # ================= attention per head =================
            with ExitStack() as p2:
                apool = p2.enter_context(tc.tile_pool(name="apool", bufs=2))
                epool = p2.enter_context(tc.tile_pool(name="epool", bufs=2))
                opool = p2.enter_context(tc.tile_pool(name="opool", bufs=3))
                psS = p2.enter_context(tc.tile_pool(name="psS", bufs=2, space="PSUM"))
                psA = p2.enter_context(tc.tile_pool(name="psA", bufs=2, space="PSUM"))

                for h in range(HPC):
                    hs = slice(h * 128, (h + 1) * 128)
                    qri = apool.tile([128, S], F32R, tag="qri")
                    q2 = apool.tile([128, S], F32R, tag="q2")
                    kri = apool.tile([128, S], F32R, tag="kri")
                    vsb = apool.tile([128, 8, 128], F32R, tag="v")
                    nc.sync.dma_start(out=qri, in_=qriT_d.ap()[hs, :])
                    nc.sync.dma_start(out=q2, in_=q2T_d.ap()[hs, :])
                    nc.sync.dma_start(out=kri, in_=kriT_d.ap()[hs, :])
                    nc.sync.dma_start(out=vsb, in_=v_d.ap()[:, hs].rearrange("(kt p) f -> p kt f", p=128))

                    for qc in range(2):
                        qs = slice(qc * 512, (qc + 1) * 512)
                        avp = psA.tile([128, 512], F32, tag="av")
                        rbp = psA.tile([128, 512], F32, tag="rb")
                        nkt = 4 * (qc + 1)
                        for kt in range(nkt):
                            lhsT = kri[:, kt * 128:(kt + 1) * 128]
                            sre = psS.tile([128, 512], F32, tag="sre")
                            sim = psS.tile([128, 512], F32, tag="sim")
                            nc.tensor.matmul(sre, lhsT, qri[:, qs], start=True, stop=True)
                            nc.tensor.matmul(sim, lhsT, q2[:, qs], start=True, stop=True)
                            c1 = epool.tile([128, 512], F32, tag="c1")
                            nc.vector.tensor_copy(c1, sre)
                            t = epool.tile([128, 512], F32, tag="t")
                            nc.vector.tensor_mul(t, c1, sre)
                            c2 = epool.tile([128, 512], F32, tag="c2")
                            nc.vector.tensor_copy(c2, sim)
                            t2 = epool.tile([128, 512], F32, tag="t2")
                            nc.vector.tensor_mul(t2, c2, sim)
                            u = epool.tile([128, 512], F32, tag="u")
                            nc.gpsimd.tensor_add(u, t, t2)
                            # logit = sqrt(u)/sqrt(HD) = exp(0.5 ln u - 0.5 ln HD);
                            # Ln and Exp share one ACT table set (no reloads)
                            w1 = epool.tile([128, 512], F32, tag="w1")
                            nc.scalar.activation(w1, u, mybir.ActivationFunctionType.Ln)
                            u2 = epool.tile([128, 512], F32, tag="u2")
                            nc.scalar.activation(u2, w1, mybir.ActivationFunctionType.Exp,
                                                 bias=nln8, scale=0.5)
                            o = kt * 128 - qc * 512
                            if o >= 0:  # diagonal-crossing block: mask cols < k
                                nc.vector.tensor_add(u2[:, 0:o + 128], u2[:, 0:o + 128],
                                                     mask_sb[:, 512 - o:640])
                            e = epool.tile([128, 512], F32R, tag="e")
                            nc.scalar.activation(e, u2, mybir.ActivationFunctionType.Exp)
                            nc.tensor.matmul(avp, vsb[:, kt, :], e,
                                             start=(kt == 0), stop=(kt == nkt - 1))
                            nc.tensor.matmul(rbp, ones_sb, e,
                                             start=(kt == 0), stop=(kt == nkt - 1))
                        rr = epool.tile([128, 512], F32, tag="rr")
                        nc.vector.reciprocal(rr, rbp)
                        ot = opool.tile([128, 512], F32R, tag="ot")
                        nc.vector.tensor_mul(ot, avp, rr)
                        dst = otA if h < 4 else otB
                        hh = h if h < 4 else h - 4
                        nc.sync.dma_start(out=dst.ap()[hh * 128:(hh + 1) * 128, qs], in_=ot)

                    if h == 3:
                        nc.gpsimd.collective_compute(
                            "AllGather", mybir.AluOpType.bypass,
                            ins=[otA.ap()], outs=[gA.ap()], replica_groups=groups)
                if True:
                    nc.gpsimd.collective_compute(
                        "AllGather", mybir.AluOpType.bypass,
                        ins=[otB.ap()], outs=[gB.ap()], replica_groups=groups)

            # ================= output projection =================
            with ExitStack() as p3:
                ppool = p3.enter_context(tc.tile_pool(name="ppool", bufs=1))
                lpool = p3.enter_context(tc.tile_pool(name="lpool", bufs=2))
                ypool = p3.enter_context(tc.tile_pool(name="ypool", bufs=3))
                psP = p3.enter_context(tc.tile_pool(name="psP", bufs=2, space="PSUM"))

                wpr_sb = ppool.tile([128, 16, 512], F32R)
                wpi_sb = ppool.tile([128, 16, 512], F32R)
                nc.sync.dma_start(out=wpr_sb, in_=Wpr.rearrange("(fc p) c -> p fc c", p=128))
                nc.sync.dma_start(out=wpi_sb, in_=Wpi.rearrange("(fc p) c -> p fc c", p=128))

                for m in range(8):
                    ms = slice(m * 128, (m + 1) * 128)
                    lha = lpool.tile([128, 8, 128], F32R, tag="lha")
                    lhb = lpool.tile([128, 8, 128], F32R, tag="lhb")
                    nc.sync.dma_start(out=lha, in_=gA.ap()[:, ms].rearrange("(fc p) s -> p fc s", p=128))
                    nc.sync.dma_start(out=lhb, in_=gB.ap()[:, ms].rearrange("(fc p) s -> p fc s", p=128))
                    pyr = psP.tile([128, 512], F32, tag="yr")
                    pyi = psP.tile([128, 512], F32, tag="yi")
                    for fc in range(16):
                        lh = lha[:, fc, :] if fc < 8 else lhb[:, fc - 8, :]
                        nc.tensor.matmul(pyr, lh, wpr_sb[:, fc, :],
                                         start=(fc == 0), stop=(fc == 15))
                        nc.tensor.matmul(pyi, lh, wpi_sb[:, fc, :],
                                         start=(fc == 0), stop=(fc == 15))
                    syr = ypool.tile([128, 512], F32, tag="syr")
                    nc.scalar.copy(syr, pyr)
                    nc.sync.dma_start(out=y[0, ms, :], in_=syr)
                    syi = ypool.tile([128, 512], F32, tag="syi")
                    nc.vector.tensor_copy(syi, pyi)
                    nc.sync.dma_start(out=y[1, ms, :], in_=syi)

    nc.compile()
    return nc


def _prep_inputs(x_re, x_im, wqkv_re, wqkv_im, wproj_re, wproj_im):
    """Pack per-core input maps (all host-side numpy)."""
    f32 = np.float32

    def qkv_rows(kind, g):
        off = {"q": 0, "k": 1024, "v": 2048}[kind]
        return slice(off + g * 64, off + (g + 1) * 64)

    # mask: cols 0-511 = NEG; cols 512+jj: 0 if jj >= k else NEG
    maskf = np.full((128, 640), NEG, f32)
    k_idx = np.arange(128)[:, None]
    jj = np.arange(128)[None, :]
    maskf[:, 512:] = np.where(jj >= k_idx, 0.0, NEG)

    WA_hg, WB_hg, Wpr_hg, Wpi_hg = {}, {}, {}, {}
    fc_order = [0, 1, 2, 3, 8, 9, 10, 11, 4, 5, 6, 7, 12, 13, 14, 15]
    for hg in range(2):
        WA = np.empty((D, 4096), f32)
        WB = np.empty((D, 4096), f32)
        for h in range(HPC):
            g = hg * HPC + h
            Wqr = wqkv_re[qkv_rows("q", g)]   # [64, D]
            Wqi = wqkv_im[qkv_rows("q", g)]
            Wkr = wqkv_re[qkv_rows("k", g)]
            Wki = wqkv_im[qkv_rows("k", g)]
            Wvr = wqkv_re[qkv_rows("v", g)]
            Wvi = wqkv_im[qkv_rows("v", g)]
            c = h * 128
            # qriT = [qr; qi]
            WA[:, c:c + 64] = Wqr.T;        WA[:, c + 64:c + 128] = Wqi.T
            WB[:, c:c + 64] = -Wqi.T;       WB[:, c + 64:c + 128] = Wqr.T
            # q2T = [qi; -qr]
            c = 1024 + h * 128
            WA[:, c:c + 64] = Wqi.T;        WA[:, c + 64:c + 128] = -Wqr.T
            WB[:, c:c + 64] = Wqr.T;        WB[:, c + 64:c + 128] = Wqi.T
            # kriT = [kr; ki]
            c = 2048 + h * 128
            WA[:, c:c + 64] = Wkr.T;        WA[:, c + 64:c + 128] = Wki.T
            WB[:, c:c + 64] = -Wki.T;       WB[:, c + 64:c + 128] = Wkr.T
            # v natural = [vr; vi]
            c = 3072 + h * 128
            WA[:, c:c + 64] = Wvr.T;        WA[:, c + 64:c + 128] = Wvi.T
            WB[:, c:c + 64] = -Wvi.T;       WB[:, c + 64:c + 128] = Wvr.T
        WA_hg[hg], WB_hg[hg] = np.ascontiguousarray(WA), np.ascontiguousarray(WB)

        cols = slice(hg * 512, (hg + 1) * 512)
        Wpr = np.empty((2048, 512), f32)
        Wpi = np.empty((2048, 512), f32)
        for fci, g in enumerate(fc_order):
            gs = slice(g * 64, (g + 1) * 64)
            r = fci * 128
            Wpr[r:r + 64] = wproj_re[cols, gs].T
            Wpr[r + 64:r + 128] = -wproj_im[cols, gs].T
            Wpi[r:r + 64] = wproj_im[cols, gs].T
            Wpi[r + 64:r + 128] = wproj_re[cols, gs].T
        Wpr_hg[hg], Wpi_hg[hg] = np.ascontiguousarray(Wpr), np.ascontiguousarray(Wpi)

    xT = {}
    for b in range(B):
        xT[b] = (np.ascontiguousarray(x_re[b].T.astype(f32)),
                 np.ascontiguousarray(x_im[b].T.astype(f32)))

    in_maps = []
    for c in range(N_CORES):
        b, hg = c // 2, c % 2
        in_maps.append({
            "xrT": xT[b][0], "xiT": xT[b][1],
            "WA": WA_hg[hg], "WB": WB_hg[hg],
            "Wpr": Wpr_hg[hg], "Wpi": Wpi_hg[hg],
            "maskf": maskf,
        })
    return in_maps


def _get_nc():
    if "nc" not in _CACHE:
        _CACHE["nc"] = _build()
    return _CACHE["nc"]


def kernel(x_re, x_im, wqkv_re, wqkv_im, wproj_re, wproj_im, _trace=False):
    nc = _get_nc()
    in_maps = _prep_inputs(np.asarray(x_re, np.float32), np.asarray(x_im, np.float32),
                           np.asarray(wqkv_re, np.float32), np.asarray(wqkv_im, np.float32),
                           np.asarray(wproj_re, np.float32), np.asarray(wproj_im, np.float32))
    res = run_bass_kernel_spmd(nc, in_maps, list(range(N_CORES)), trace=_trace)
    out = np.empty((2, B, S, D), np.float32)
    for c in range(N_CORES):
        b, hg = c // 2, c % 2
        yc = res.results[c]["y"]
        out[0, b, :, hg * 512:(hg + 1) * 512] = yc[0]
        out[1, b, :, hg * 512:(hg + 1) * 512] = yc[1]
    if _trace:
        return out, res
    return out


# revision 8
# speedup vs baseline: 1.2439x; 1.0374x over previous
"""Complex attention kernel for 8 TRN2 NeuronCores (SPMD).

Sharding: core c -> batch b=c//2, head-group hg=c%2 (8 of 16 heads).
Stage 1 computes q/k (transposed layout, complex parts packed along
partitions with sign folded into host-packed weights), v in natural
layout. Attention works on s^T[k,q] blocks so no on-chip transposes are
needed; softmax runs without max-subtraction (logits = |s|/8 >= 0).
A pairwise AllGather exchanges attention outputs before the output
projection; each core computes half the projection columns.

All matmuls in float32r (full PE rate at N>=256, ~1e-4 matmul error).
"""
from contextlib import ExitStack

import numpy as np

import concourse.bass as bass
import concourse.tile as tile
from concourse import bacc, mybir
from concourse.bass_utils import run_bass_kernel_spmd

B, S, D, H = 4, 1024, 1024, 16
HD = 64          # head dim
HPC = 8          # heads per core
N_CORES = 8
NEG = -300.0     # mask bias: exp(u + NEG) == 0 in fp32

F32 = mybir.dt.float32
F32R = mybir.dt.float32r
BF16 = mybir.dt.bfloat16
import os as _os
import ml_dtypes as _mld

def _dt(name, default):
    v = _os.environ.get(name, default)
    return {"f32r": F32R, "bf16": BF16, "f32": F32}[v]

DT_S1 = _dt("KDT_S1", "bf16")   # x, Wqkv packed (stage-1 matmul operands)
DT_QK = _dt("KDT_QK", "bf16")   # q/k transposed storage (score matmuls)
DT_AV = _dt("KDT_AV", "bf16")   # v, e, ones (av + rowsum matmuls)
DT_OT = _dt("KDT_OT", "bf16")   # attention out, Wproj (proj matmuls)

def _np_of(dt):
    return _mld.bfloat16 if dt == BF16 else np.float32

_CACHE = {}


def _patch_act_tables():
    """Make natural_log_exp_and_others the only set containing Ln/Exp so the
    act-table-load pass keeps one table set resident through the attention
    phase (instead of ping-ponging exp_and_others <-> natural_log, ~2.7us
    per reload). Only set *contents* are filtered; set order/indices are
    unchanged, so act_func_set_id stays valid for walrus."""
    if _CACHE.get("act_patched"):
        return
    import concourse.bacc as _bacc
    import concourse.hw_specs as _hw
    orig = _hw.get_activation_tables

    def patched(arch):
        tabs = dict(orig(arch))
        out = {}
        for name, fns in tabs.items():
            if name != "natural_log_exp_and_others":
                fns = {f for f in fns
                       if f not in (mybir.ActivationFunctionType.Exp,
                                    mybir.ActivationFunctionType.Ln)}
            out[name] = fns
        return out

    _bacc.get_activation_tables = patched
    _CACHE["act_patched"] = True


def _build():
    _patch_act_tables()
    nc = bacc.Bacc("TRN2", target_bir_lowering=False, debug=False, num_devices=N_CORES)

    # ---- I/O ----
    xrT = nc.dram_tensor("xrT", [D, S], DT_S1, kind="ExternalInput").ap()
    xiT = nc.dram_tensor("xiT", [D, S], DT_S1, kind="ExternalInput").ap()
    WA = nc.dram_tensor("WA", [D, 4 * 1024], DT_S1, kind="ExternalInput").ap()
    WB = nc.dram_tensor("WB", [D, 4 * 1024], DT_S1, kind="ExternalInput").ap()
    Wpr = nc.dram_tensor("Wpr", [2048, 512], DT_OT, kind="ExternalInput").ap()
    Wpi = nc.dram_tensor("Wpi", [2048, 512], DT_OT, kind="ExternalInput").ap()
    maskf = nc.dram_tensor("maskf", [128, 640], F32, kind="ExternalInput").ap()
    y = nc.dram_tensor("y", [2, S, 512], F32, kind="ExternalOutput").ap()

    # ---- internal DRAM ----
    qriT_d = nc.dram_tensor("qriT_d", [1024, S], DT_QK)
    q2T_d = nc.dram_tensor("q2T_d", [1024, S], DT_QK)
    kriT_d = nc.dram_tensor("kriT_d", [1024, S], DT_QK)
    v_d = nc.dram_tensor("v_d", [S, 1024], DT_AV)
    otA = nc.dram_tensor("otA", [512, S], DT_OT)   # heads 0-3 outT
    otB = nc.dram_tensor("otB", [512, S], DT_OT)   # heads 4-7 outT
    gA = nc.dram_tensor("gA", [1024, S], DT_OT)    # gathered heads {0-3, 8-11}
    gB = nc.dram_tensor("gB", [1024, S], DT_OT)    # gathered heads {4-7, 12-15}

    groups = [[0, 1], [2, 3], [4, 5], [6, 7]]

    with tile.TileContext(nc) as tc:
        with ExitStack() as ctx:
            singles = ctx.enter_context(tc.tile_pool(name="singles", bufs=1))
            mask_sb = singles.tile([128, 640], F32)
            nc.sync.dma_start(out=mask_sb, in_=maskf)
            ones32 = singles.tile([128, 128], F32)
            nc.vector.memset(ones32, 1.0)
            ones_sb = singles.tile([128, 128], DT_AV)
            nc.vector.tensor_copy(ones_sb, ones32)
            nln8 = singles.tile([128, 1], F32)
            nc.vector.memset(nln8, -0.5 * np.log(float(HD)))  # exp(0.5 ln t + this) = sqrt(t)/sqrt(HD)

            # ================= stage 1: qkv projections =================
            with ExitStack() as p1:
                xpool = p1.enter_context(tc.tile_pool(name="xpool", bufs=1))
                wpool = p1.enter_context(tc.tile_pool(name="wpool", bufs=2))
                spool = p1.enter_context(tc.tile_pool(name="spool", bufs=3))
                ps1 = p1.enter_context(tc.tile_pool(name="ps1", bufs=2, space="PSUM"))
                psv = p1.enter_context(tc.tile_pool(name="psv", bufs=4, space="PSUM"))

                xr_sb = xpool.tile([128, 8, S], DT_S1)
                xi_sb = xpool.tile([128, 8, S], DT_S1)
                nc.sync.dma_start(out=xr_sb, in_=xrT.rearrange("(dc p) s -> p dc s", p=128))
                nc.sync.dma_start(out=xi_sb, in_=xiT.rearrange("(dc p) s -> p dc s", p=128))

                # v first (natural layout), so per-head attention can start early
                for cc in range(2):
                    wv = wpool.tile([128, 8, 512], DT_S1, tag="wv")
                    wvb = wpool.tile([128, 8, 512], DT_S1, tag="wvb")
                    cb = 3072 + cc * 512
                    nc.sync.dma_start(out=wv, in_=WA[:, cb:cb + 512].rearrange("(dc p) f -> p dc f", p=128))
                    nc.sync.dma_start(out=wvb, in_=WB[:, cb:cb + 512].rearrange("(dc p) f -> p dc f", p=128))
                    for m in range(8):
                        ps = psv.tile([128, 512], F32, tag="v")
                        ms = slice(m * 128, (m + 1) * 128)
                        for dc in range(8):
                            nc.tensor.matmul(ps, xr_sb[:, dc, ms], wv[:, dc, :],
                                             start=(dc == 0), stop=False)
                            nc.tensor.matmul(ps, xi_sb[:, dc, ms], wvb[:, dc, :],
                                             start=False, stop=(dc == 7))
                        st = spool.tile([128, 512], DT_AV, tag="stv")
                        nc.scalar.copy(st, ps)
                        nc.sync.dma_start(out=v_d.ap()[ms, cc * 512:(cc + 1) * 512], in_=st)

                # q/q2/k transposed layout, grouped per head
                for h in range(HPC):
                    for cb, dest in ((h * 128, qriT_d), (1024 + h * 128, q2T_d),
                                     (2048 + h * 128, kriT_d)):
                        wa = wpool.tile([128, 8, 128], DT_S1, tag="wa")
                        wb = wpool.tile([128, 8, 128], DT_S1, tag="wb")
                        nc.sync.dma_start(out=wa, in_=WA[:, cb:cb + 128].rearrange("(dc p) f -> p dc f", p=128))
                        nc.sync.dma_start(out=wb, in_=WB[:, cb:cb + 128].rearrange("(dc p) f -> p dc f", p=128))
                        ps0 = ps1.tile([128, 512], F32, tag="t0")
                        psx = ps1.tile([128, 512], F32, tag="t1")
                        for dc in range(8):
                            nc.tensor.matmul(ps0, wa[:, dc, :], xr_sb[:, dc, 0:512],
                                             start=(dc == 0), stop=False)
                            nc.tensor.matmul(psx, wa[:, dc, :], xr_sb[:, dc, 512:1024],
                                             start=(dc == 0), stop=False)
                            nc.tensor.matmul(ps0, wb[:, dc, :], xi_sb[:, dc, 0:512],
                                             start=False, stop=(dc == 7))
                            nc.tensor.matmul(psx, wb[:, dc, :], xi_sb[:, dc, 512:1024],
                                             start=False, stop=(dc == 7))
                        st = spool.tile([128, 1024], DT_QK, tag="st")
                        nc.scalar.copy(st[:, 0:512], ps0)
                        nc.scalar.copy(st[:, 512:1024], psx)
                        hs = slice(h * 128, (h + 1) * 128)
                        nc.sync.dma_start(out=dest.ap()[hs, :], in_=st)

            # ================= attention per head =================
            with ExitStack() as p2:
                apool = p2.enter_context(tc.tile_pool(name="apool", bufs=2))
                epool = p2.enter_context(tc.tile_pool(name="epool", bufs=2))
                opool = p2.enter_context(tc.tile_pool(name="opool", bufs=3))
                psS = p2.enter_context(tc.tile_pool(name="psS", bufs=2, space="PSUM"))
                psA = p2.enter_context(tc.tile_pool(name="psA", bufs=2, space="PSUM"))

                for h in range(HPC):
                    hs = slice(h * 128, (h + 1) * 128)
                    qri = apool.tile([128, S], DT_QK, tag="qri")
                    q2 = apool.tile([128, S], DT_QK, tag="q2")
                    kri = apool.tile([128, S], DT_QK, tag="kri")
                    vsb = apool.tile([128, 8, 128], DT_AV, tag="v")
                    nc.sync.dma_start(out=qri, in_=qriT_d.ap()[hs, :])
                    nc.sync.dma_start(out=q2, in_=q2T_d.ap()[hs, :])
                    nc.sync.dma_start(out=kri, in_=kriT_d.ap()[hs, :])
                    nc.sync.dma_start(out=vsb, in_=v_d.ap()[:, hs].rearrange("(kt p) f -> p kt f", p=128))

                    for qc in range(2):
                        qs = slice(qc * 512, (qc + 1) * 512)
                        avp = psA.tile([128, 512], F32, tag="av")
                        rbp = psA.tile([128, 512], F32, tag="rb")
                        nkt = 4 * (qc + 1)
                        for kt in range(nkt):
                            lhsT = kri[:, kt * 128:(kt + 1) * 128]
                            sre = psS.tile([128, 512], F32, tag="sre")
                            sim = psS.tile([128, 512], F32, tag="sim")
                            nc.tensor.matmul(sre, lhsT, qri[:, qs], start=True, stop=True)
                            nc.tensor.matmul(sim, lhsT, q2[:, qs], start=True, stop=True)
                            c1 = epool.tile([128, 512], F32, tag="c1")
                            nc.vector.tensor_copy(c1, sre)
                            t = epool.tile([128, 512], F32, tag="t")
                            nc.vector.tensor_mul(t, c1, sre)
                            c2 = epool.tile([128, 512], F32, tag="c2")
                            nc.vector.tensor_copy(c2, sim)
                            t2 = epool.tile([128, 512], F32, tag="t2")
                            nc.vector.tensor_mul(t2, c2, sim)
                            u = epool.tile([128, 512], F32, tag="u")
                            nc.gpsimd.tensor_add(u, t, t2)
                            # logit = sqrt(u)/sqrt(HD) = exp(0.5 ln u - 0.5 ln HD);
                            # Ln and Exp share one ACT table set (no reloads)
                            w1 = epool.tile([128, 512], F32, tag="w1")
                            nc.scalar.activation(w1, u, mybir.ActivationFunctionType.Ln)
                            u2 = epool.tile([128, 512], F32, tag="u2")
                            nc.scalar.activation(u2, w1, mybir.ActivationFunctionType.Exp,
                                                 bias=nln8, scale=0.5)
                            o = kt * 128 - qc * 512
                            if o >= 0:  # diagonal-crossing block: mask cols < k
                                nc.vector.tensor_add(u2[:, 0:o + 128], u2[:, 0:o + 128],
                                                     mask_sb[:, 512 - o:640])
                            e = epool.tile([128, 512], DT_AV, tag="e")
                            nc.scalar.activation(e, u2, mybir.ActivationFunctionType.Exp)
                            nc.tensor.matmul(avp, vsb[:, kt, :], e,
                                             start=(kt == 0), stop=(kt == nkt - 1))
                            nc.tensor.matmul(rbp, ones_sb, e,
                                             start=(kt == 0), stop=(kt == nkt - 1))
                        rr = epool.tile([128, 512], F32, tag="rr")
                        nc.vector.reciprocal(rr, rbp)
                        ot = opool.tile([128, 512], DT_OT, tag="ot")
                        nc.vector.tensor_mul(ot, avp, rr)
                        dst = otA if h < 4 else otB
                        hh = h if h < 4 else h - 4
                        nc.sync.dma_start(out=dst.ap()[hh * 128:(hh + 1) * 128, qs], in_=ot)

                    if h == 3:
                        nc.gpsimd.collective_compute(
                            "AllGather", mybir.AluOpType.bypass,
                            ins=[otA.ap()], outs=[gA.ap()], replica_groups=groups)
                if True:
                    nc.gpsimd.collective_compute(
                        "AllGather", mybir.AluOpType.bypass,
                        ins=[otB.ap()], outs=[gB.ap()], replica_groups=groups)

            # ================= output projection =================
            with ExitStack() as p3:
                ppool = p3.enter_context(tc.tile_pool(name="ppool", bufs=1))
                lpool = p3.enter_context(tc.tile_pool(name="lpool", bufs=2))
                ypool = p3.enter_context(tc.tile_pool(name="ypool", bufs=3))
                psP = p3.enter_context(tc.tile_pool(name="psP", bufs=2, space="PSUM"))

                wpr_sb = ppool.tile([128, 16, 512], DT_OT)
                wpi_sb = ppool.tile([128, 16, 512], DT_OT)
                nc.sync.dma_start(out=wpr_sb, in_=Wpr.rearrange("(fc p) c -> p fc c", p=128))
                nc.sync.dma_start(out=wpi_sb, in_=Wpi.rearrange("(fc p) c -> p fc c", p=128))

                for m in range(8):
                    ms = slice(m * 128, (m + 1) * 128)
                    lha = lpool.tile([128, 8, 128], DT_OT, tag="lha")
                    lhb = lpool.tile([128, 8, 128], DT_OT, tag="lhb")
                    nc.sync.dma_start(out=lha, in_=gA.ap()[:, ms].rearrange("(fc p) s -> p fc s", p=128))
                    nc.sync.dma_start(out=lhb, in_=gB.ap()[:, ms].rearrange("(fc p) s -> p fc s", p=128))
                    pyr = psP.tile([128, 512], F32, tag="yr")
                    pyi = psP.tile([128, 512], F32, tag="yi")
                    for fc in range(16):
                        lh = lha[:, fc, :] if fc < 8 else lhb[:, fc - 8, :]
                        nc.tensor.matmul(pyr, lh, wpr_sb[:, fc, :],
                                         start=(fc == 0), stop=(fc == 15))
                        nc.tensor.matmul(pyi, lh, wpi_sb[:, fc, :],
                                         start=(fc == 0), stop=(fc == 15))
                    syr = ypool.tile([128, 512], F32, tag="syr")
                    nc.scalar.copy(syr, pyr)
                    nc.sync.dma_start(out=y[0, ms, :], in_=syr)
                    syi = ypool.tile([128, 512], F32, tag="syi")
                    nc.vector.tensor_copy(syi, pyi)
                    nc.sync.dma_start(out=y[1, ms, :], in_=syi)

    nc.compile()
    return nc


def _prep_inputs(x_re, x_im, wqkv_re, wqkv_im, wproj_re, wproj_im):
    """Pack per-core input maps (all host-side numpy)."""
    f32 = np.float32

    def qkv_rows(kind, g):
        off = {"q": 0, "k": 1024, "v": 2048}[kind]
        return slice(off + g * 64, off + (g + 1) * 64)

    # mask: cols 0-511 = NEG; cols 512+jj: 0 if jj >= k else NEG
    maskf = np.full((128, 640), NEG, f32)
    k_idx = np.arange(128)[:, None]
    jj = np.arange(128)[None, :]
    maskf[:, 512:] = np.where(jj >= k_idx, 0.0, NEG)

    WA_hg, WB_hg, Wpr_hg, Wpi_hg = {}, {}, {}, {}
    fc_order = [0, 1, 2, 3, 8, 9, 10, 11, 4, 5, 6, 7, 12, 13, 14, 15]
    for hg in range(2):
        WA = np.empty((D, 4096), f32)
        WB = np.empty((D, 4096), f32)
        for h in range(HPC):
            g = hg * HPC + h
            Wqr = wqkv_re[qkv_rows("q", g)]   # [64, D]
            Wqi = wqkv_im[qkv_rows("q", g)]
            Wkr = wqkv_re[qkv_rows("k", g)]
            Wki = wqkv_im[qkv_rows("k", g)]
            Wvr = wqkv_re[qkv_rows("v", g)]
            Wvi = wqkv_im[qkv_rows("v", g)]
            c = h * 128
            # qriT = [qr; qi]
            WA[:, c:c + 64] = Wqr.T;        WA[:, c + 64:c + 128] = Wqi.T
            WB[:, c:c + 64] = -Wqi.T;       WB[:, c + 64:c + 128] = Wqr.T
            # q2T = [qi; -qr]
            c = 1024 + h * 128
            WA[:, c:c + 64] = Wqi.T;        WA[:, c + 64:c + 128] = -Wqr.T
            WB[:, c:c + 64] = Wqr.T;        WB[:, c + 64:c + 128] = Wqi.T
            # kriT = [kr; ki]
            c = 2048 + h * 128
            WA[:, c:c + 64] = Wkr.T;        WA[:, c + 64:c + 128] = Wki.T
            WB[:, c:c + 64] = -Wki.T;       WB[:, c + 64:c + 128] = Wkr.T
            # v natural = [vr; vi]
            c = 3072 + h * 128
            WA[:, c:c + 64] = Wvr.T;        WA[:, c + 64:c + 128] = Wvi.T
            WB[:, c:c + 64] = -Wvi.T;       WB[:, c + 64:c + 128] = Wvr.T
        WA_hg[hg], WB_hg[hg] = np.ascontiguousarray(WA), np.ascontiguousarray(WB)

        cols = slice(hg * 512, (hg + 1) * 512)
        Wpr = np.empty((2048, 512), f32)
        Wpi = np.empty((2048, 512), f32)
        for fci, g in enumerate(fc_order):
            gs = slice(g * 64, (g + 1) * 64)
            r = fci * 128
            Wpr[r:r + 64] = wproj_re[cols, gs].T
            Wpr[r + 64:r + 128] = -wproj_im[cols, gs].T
            Wpi[r:r + 64] = wproj_im[cols, gs].T
            Wpi[r + 64:r + 128] = wproj_re[cols, gs].T
        Wpr_hg[hg], Wpi_hg[hg] = np.ascontiguousarray(Wpr), np.ascontiguousarray(Wpi)

    xT = {}
    for b in range(B):
        xT[b] = (np.ascontiguousarray(x_re[b].T.astype(f32)),
                 np.ascontiguousarray(x_im[b].T.astype(f32)))

    in_maps = []
    for c in range(N_CORES):
        b, hg = c // 2, c % 2
        in_maps.append({
            "xrT": xT[b][0], "xiT": xT[b][1],
            "WA": WA_hg[hg], "WB": WB_hg[hg],
            "Wpr": Wpr_hg[hg], "Wpi": Wpi_hg[hg],
            "maskf": maskf,
        })
    # convert to the DRAM dtypes the kernel declares (cached per id)
    np_s1, np_ot = _np_of(DT_S1), _np_of(DT_OT)
    conv = {}
    def cv(a, npdt):
        key = (id(a), np.dtype(npdt).name)
        if key not in conv:
            conv[key] = np.ascontiguousarray(a.astype(npdt))
        return conv[key]
    for im in in_maps:
        for k in ("xrT", "xiT", "WA", "WB"):
            im[k] = cv(im[k], np_s1)
        for k in ("Wpr", "Wpi"):
            im[k] = cv(im[k], np_ot)
    return in_maps


def _get_nc():
    if "nc" not in _CACHE:
        _CACHE["nc"] = _build()
    return _CACHE["nc"]


def kernel(x_re, x_im, wqkv_re, wqkv_im, wproj_re, wproj_im, _trace=False):
    nc = _get_nc()
    in_maps = _prep_inputs(np.asarray(x_re, np.float32), np.asarray(x_im, np.float32),
                           np.asarray(wqkv_re, np.float32), np.asarray(wqkv_im, np.float32),
                           np.asarray(wproj_re, np.float32), np.asarray(wproj_im, np.float32))
    res = run_bass_kernel_spmd(nc, in_maps, list(range(N_CORES)), trace=_trace)
    out = np.empty((2, B, S, D), np.float32)
    for c in range(N_CORES):
        b, hg = c // 2, c % 2
        yc = res.results[c]["y"]
        out[0, b, :, hg * 512:(hg + 1) * 512] = yc[0]
        out[1, b, :, hg * 512:(hg + 1) * 512] = yc[1]
    if _trace:
        return out, res
    return out


# revision 10
# speedup vs baseline: 1.6172x; 1.3001x over previous
"""Complex attention kernel for 8 TRN2 NeuronCores (SPMD).

Sharding: core c -> batch b=c//2, head-group hg=c%2 (8 of 16 heads).
Stage 1 computes q/k (transposed layout, complex parts packed along
partitions with sign folded into host-packed weights), v in natural
layout. Attention works on s^T[k,q] blocks so no on-chip transposes are
needed; softmax runs without max-subtraction (logits = |s|/8 >= 0).
A pairwise AllGather exchanges attention outputs before the output
projection; each core computes half the projection columns.

All matmuls in float32r (full PE rate at N>=256, ~1e-4 matmul error).
"""
from contextlib import ExitStack

import numpy as np

import concourse.bass as bass
import concourse.tile as tile
from concourse import bacc, mybir
from concourse.bass_utils import run_bass_kernel_spmd

B, S, D, H = 4, 1024, 1024, 16
HD = 64          # head dim
HPC = 8          # heads per core
N_CORES = 8
NEG = -300.0     # mask bias: exp(u + NEG) == 0 in fp32

F32 = mybir.dt.float32
F32R = mybir.dt.float32r
BF16 = mybir.dt.bfloat16
import os as _os
import ml_dtypes as _mld

def _dt(name, default):
    v = _os.environ.get(name, default)
    return {"f32r": F32R, "bf16": BF16, "f32": F32}[v]

DT_S1 = _dt("KDT_S1", "f32r")   # x, Wqkv packed (stage-1 matmul operands)
DT_QK = _dt("KDT_QK", "f32r")   # q/k transposed storage (score matmuls)
DT_AV = _dt("KDT_AV", "bf16")   # v, e, ones (av + rowsum matmuls)
DT_OT = _dt("KDT_OT", "bf16")   # attention out, Wproj (proj matmuls)

def _np_of(dt):
    return _mld.bfloat16 if dt == BF16 else np.float32

_CACHE = {}


def _patch_act_tables():
    """Make natural_log_exp_and_others the only set containing Ln/Exp so the
    act-table-load pass keeps one table set resident through the attention
    phase (instead of ping-ponging exp_and_others <-> natural_log, ~2.7us
    per reload). Only set *contents* are filtered; set order/indices are
    unchanged, so act_func_set_id stays valid for walrus."""
    if _CACHE.get("act_patched"):
        return
    import concourse.bacc as _bacc
    import concourse.hw_specs as _hw
    orig = _hw.get_activation_tables

    def patched(arch):
        tabs = dict(orig(arch))
        out = {}
        for name, fns in tabs.items():
            if name != "natural_log_exp_and_others":
                fns = {f for f in fns
                       if f not in (mybir.ActivationFunctionType.Exp,
                                    mybir.ActivationFunctionType.Ln)}
            out[name] = fns
        return out

    _bacc.get_activation_tables = patched
    _CACHE["act_patched"] = True


def _build():
    _patch_act_tables()
    nc = bacc.Bacc("TRN2", target_bir_lowering=False, debug=False, num_devices=N_CORES)

    # ---- I/O ----
    xrT = nc.dram_tensor("xrT", [D, S], DT_S1, kind="ExternalInput").ap()
    xiT = nc.dram_tensor("xiT", [D, S], DT_S1, kind="ExternalInput").ap()
    WA = nc.dram_tensor("WA", [D, 4 * 1024], DT_S1, kind="ExternalInput").ap()
    WB = nc.dram_tensor("WB", [D, 4 * 1024], DT_S1, kind="ExternalInput").ap()
    Wpr = nc.dram_tensor("Wpr", [2048, 512], DT_OT, kind="ExternalInput").ap()
    Wpi = nc.dram_tensor("Wpi", [2048, 512], DT_OT, kind="ExternalInput").ap()
    maskf = nc.dram_tensor("maskf", [128, 640], F32, kind="ExternalInput").ap()
    y = nc.dram_tensor("y", [2, S, 512], F32, kind="ExternalOutput").ap()

    # ---- internal DRAM ----
    qriT_d = nc.dram_tensor("qriT_d", [1024, S], DT_QK)
    kriT_d = nc.dram_tensor("kriT_d", [1024, S], DT_QK)
    v_d = nc.dram_tensor("v_d", [S, 1024], DT_AV)
    otA = nc.dram_tensor("otA", [512, S], DT_OT)   # heads 0-3 outT
    otB = nc.dram_tensor("otB", [512, S], DT_OT)   # heads 4-7 outT
    gA = nc.dram_tensor("gA", [1024, S], DT_OT)    # gathered heads {0-3, 8-11}
    gB = nc.dram_tensor("gB", [1024, S], DT_OT)    # gathered heads {4-7, 12-15}

    groups = [[0, 1], [2, 3], [4, 5], [6, 7]]

    with tile.TileContext(nc) as tc:
        with ExitStack() as ctx:
            singles = ctx.enter_context(tc.tile_pool(name="singles", bufs=1))
            mask_sb = singles.tile([128, 640], F32)
            nc.sync.dma_start(out=mask_sb, in_=maskf)
            ones32 = singles.tile([128, 128], F32)
            nc.vector.memset(ones32, 1.0)
            ones_sb = singles.tile([128, 128], DT_AV)
            nc.vector.tensor_copy(ones_sb, ones32)
            nln8 = singles.tile([128, 1], F32)
            nc.vector.memset(nln8, -0.5 * np.log(float(HD)))  # exp(0.5 ln t + this) = sqrt(t)/sqrt(HD)

            # ================= stage 1: qkv projections =================
            with ExitStack() as p1:
                xpool = p1.enter_context(tc.tile_pool(name="xpool", bufs=1))
                wpool = p1.enter_context(tc.tile_pool(name="wpool", bufs=2))
                spool = p1.enter_context(tc.tile_pool(name="spool", bufs=3))
                ps1 = p1.enter_context(tc.tile_pool(name="ps1", bufs=2, space="PSUM"))
                psv = p1.enter_context(tc.tile_pool(name="psv", bufs=4, space="PSUM"))

                xr_sb = xpool.tile([128, 8, S], DT_S1)
                xi_sb = xpool.tile([128, 8, S], DT_S1)
                for dc in range(8):
                    nc.sync.dma_start(out=xr_sb[:, dc, :], in_=xrT[dc * 128:(dc + 1) * 128, :])
                    nc.sync.dma_start(out=xi_sb[:, dc, :], in_=xiT[dc * 128:(dc + 1) * 128, :])

                # v first (natural layout), so per-head attention can start early
                for cc in range(2):
                    wv = wpool.tile([128, 8, 512], DT_S1, tag="wv")
                    wvb = wpool.tile([128, 8, 512], DT_S1, tag="wvb")
                    cb = 3072 + cc * 512
                    nc.sync.dma_start(out=wv, in_=WA[:, cb:cb + 512].rearrange("(dc p) f -> p dc f", p=128))
                    nc.sync.dma_start(out=wvb, in_=WB[:, cb:cb + 512].rearrange("(dc p) f -> p dc f", p=128))
                    for m in range(8):
                        ps = psv.tile([128, 512], F32, tag="v")
                        ms = slice(m * 128, (m + 1) * 128)
                        for dc in range(8):
                            nc.tensor.matmul(ps, xr_sb[:, dc, ms], wv[:, dc, :],
                                             start=(dc == 0), stop=False)
                            nc.tensor.matmul(ps, xi_sb[:, dc, ms], wvb[:, dc, :],
                                             start=False, stop=(dc == 7))
                        st = spool.tile([128, 512], DT_AV, tag="stv")
                        nc.scalar.copy(st, ps)
                        nc.sync.dma_start(out=v_d.ap()[ms, cc * 512:(cc + 1) * 512], in_=st)

                # q/q2/k transposed layout, grouped per head
                for h in range(HPC):
                    for cb, dest in ((h * 128, qriT_d), (2048 + h * 128, kriT_d)):
                        wa = wpool.tile([128, 8, 128], DT_S1, tag="wa")
                        wb = wpool.tile([128, 8, 128], DT_S1, tag="wb")
                        nc.sync.dma_start(out=wa, in_=WA[:, cb:cb + 128].rearrange("(dc p) f -> p dc f", p=128))
                        nc.sync.dma_start(out=wb, in_=WB[:, cb:cb + 128].rearrange("(dc p) f -> p dc f", p=128))
                        ps0 = ps1.tile([128, 512], F32, tag="t0")
                        psx = ps1.tile([128, 512], F32, tag="t1")
                        for dc in range(8):
                            nc.tensor.matmul(ps0, wa[:, dc, :], xr_sb[:, dc, 0:512],
                                             start=(dc == 0), stop=False)
                            nc.tensor.matmul(psx, wa[:, dc, :], xr_sb[:, dc, 512:1024],
                                             start=(dc == 0), stop=False)
                            nc.tensor.matmul(ps0, wb[:, dc, :], xi_sb[:, dc, 0:512],
                                             start=False, stop=(dc == 7))
                            nc.tensor.matmul(psx, wb[:, dc, :], xi_sb[:, dc, 512:1024],
                                             start=False, stop=(dc == 7))
                        st = spool.tile([128, 1024], DT_QK, tag="st")
                        nc.scalar.copy(st[:, 0:512], ps0)
                        nc.scalar.copy(st[:, 512:1024], psx)
                        hs = slice(h * 128, (h + 1) * 128)
                        nc.sync.dma_start(out=dest.ap()[hs, :], in_=st)

            # ================= attention per head =================
            with ExitStack() as p2:
                apool = p2.enter_context(tc.tile_pool(name="apool", bufs=2))
                epool = p2.enter_context(tc.tile_pool(name="epool", bufs=2))
                opool = p2.enter_context(tc.tile_pool(name="opool", bufs=3))
                psS = p2.enter_context(tc.tile_pool(name="psS", bufs=2, space="PSUM"))
                psA = p2.enter_context(tc.tile_pool(name="psA", bufs=2, space="PSUM"))

                for h in range(HPC):
                    hs = slice(h * 128, (h + 1) * 128)
                    qri = apool.tile([128, S], DT_QK, tag="qri")
                    q2 = apool.tile([128, S], DT_QK, tag="q2")
                    kri = apool.tile([128, S], DT_QK, tag="kri")
                    vsb = apool.tile([128, 8, 128], DT_AV, tag="v")
                    nc.sync.dma_start(out=qri, in_=qriT_d.ap()[hs, :])
                    nc.sync.dma_start(out=kri, in_=kriT_d.ap()[hs, :])
                    nc.sync.dma_start(out=vsb, in_=v_d.ap()[:, hs].rearrange("(kt p) f -> p kt f", p=128))
                    # q2 = [qi; -qr]: partition-swap via SBUF DMA, negate lower half
                    nc.sync.dma_start(out=q2[0:64, :], in_=qri[64:128, :])
                    nc.sync.dma_start(out=q2[64:128, :], in_=qri[0:64, :])
                    nc.vector.tensor_scalar_mul(q2[64:128, :], q2[64:128, :], -1.0)

                    for qc in range(2):
                        ce = (qc + 1) * 512
                        avp = psA.tile([128, 512], F32, tag="av")
                        rbp = psA.tile([128, 512], F32, tag="rb")
                        nkt = 4 * (qc + 1)
                        for kt in range(nkt):
                            cs_valid = max(kt * 128, qc * 512)
                            # keep chunks >=256 wide (fp32r full rate); widened
                            # left part is masked off below
                            cs = cs_valid if ce - cs_valid >= 256 else cs_valid - 128
                            w = ce - cs
                            po = cs - qc * 512      # offset in the qc psum
                            o = kt * 128 - cs       # diag offset within chunk
                            diag = cs_valid == kt * 128
                            lhsT = kri[:, kt * 128:(kt + 1) * 128]
                            sre = psS.tile([128, 512], F32, tag="sre")
                            sim = psS.tile([128, 512], F32, tag="sim")
                            nc.tensor.matmul(sre[:, :w], lhsT, qri[:, cs:ce], start=True, stop=True)
                            nc.tensor.matmul(sim[:, :w], lhsT, q2[:, cs:ce], start=True, stop=True)
                            # t = sre^2 on ACT (Square lives in the pinned table set)
                            t = epool.tile([128, 512], F32, tag="t")
                            nc.scalar.activation(t[:, :w], sre[:, :w],
                                                 mybir.ActivationFunctionType.Square)
                            # t2 = sim^2 on DVE (copy + mult; DVE can read one PSUM operand)
                            c2 = epool.tile([128, 512], F32, tag="c2")
                            nc.vector.tensor_copy(c2[:, :w], sim[:, :w])
                            t2 = epool.tile([128, 512], F32, tag="t2")
                            nc.vector.tensor_mul(t2[:, :w], c2[:, :w], sim[:, :w])
                            u = epool.tile([128, 512], F32, tag="u")
                            nc.gpsimd.tensor_add(u[:, :w], t[:, :w], t2[:, :w])
                            # logit = sqrt(u)/sqrt(HD) = exp(0.5 ln u - 0.5 ln HD);
                            # Ln and Exp share one ACT table set (no reloads)
                            w1 = epool.tile([128, 512], F32, tag="w1")
                            nc.scalar.activation(w1[:, :w], u[:, :w],
                                                 mybir.ActivationFunctionType.Ln)
                            u2 = epool.tile([128, 512], F32, tag="u2")
                            nc.scalar.activation(u2[:, :w], w1[:, :w],
                                                 mybir.ActivationFunctionType.Exp,
                                                 bias=nln8, scale=0.5)
                            if diag:  # mask invalid cols + diagonal triangle
                                nc.gpsimd.tensor_add(u2[:, 0:o + 128], u2[:, 0:o + 128],
                                                     mask_sb[:, 512 - o:640])
                            e = epool.tile([128, 512], DT_AV, tag="e")
                            nc.scalar.activation(e[:, :w], u2[:, :w],
                                                 mybir.ActivationFunctionType.Exp)
                            nc.tensor.matmul(avp[:, po:512], vsb[:, kt, :], e[:, :w],
                                             start=(kt == 0), stop=(kt == nkt - 1))
                            nc.tensor.matmul(rbp[:, po:512], ones_sb, e[:, :w],
                                             start=(kt == 0), stop=(kt == nkt - 1))
                        rr = epool.tile([128, 512], F32, tag="rr")
                        nc.vector.reciprocal(rr, rbp)
                        ot = opool.tile([128, 512], DT_OT, tag="ot")
                        nc.vector.tensor_mul(ot, avp, rr)
                        dst = otA if h < 4 else otB
                        hh = h if h < 4 else h - 4
                        nc.sync.dma_start(out=dst.ap()[hh * 128:(hh + 1) * 128, qc * 512:ce], in_=ot)

                    if h == 3:
                        nc.gpsimd.collective_compute(
                            "AllGather", mybir.AluOpType.bypass,
                            ins=[otA.ap()], outs=[gA.ap()], replica_groups=groups)
                if True:
                    nc.gpsimd.collective_compute(
                        "AllGather", mybir.AluOpType.bypass,
                        ins=[otB.ap()], outs=[gB.ap()], replica_groups=groups)

            # ================= output projection =================
            with ExitStack() as p3:
                ppool = p3.enter_context(tc.tile_pool(name="ppool", bufs=1))
                lpool = p3.enter_context(tc.tile_pool(name="lpool", bufs=2))
                ypool = p3.enter_context(tc.tile_pool(name="ypool", bufs=3))
                psP = p3.enter_context(tc.tile_pool(name="psP", bufs=2, space="PSUM"))

                wpr_sb = ppool.tile([128, 16, 512], DT_OT)
                wpi_sb = ppool.tile([128, 16, 512], DT_OT)
                nc.sync.dma_start(out=wpr_sb, in_=Wpr.rearrange("(fc p) c -> p fc c", p=128))
                nc.sync.dma_start(out=wpi_sb, in_=Wpi.rearrange("(fc p) c -> p fc c", p=128))

                for m in range(8):
                    ms = slice(m * 128, (m + 1) * 128)
                    lha = lpool.tile([128, 8, 128], DT_OT, tag="lha")
                    lhb = lpool.tile([128, 8, 128], DT_OT, tag="lhb")
                    nc.sync.dma_start(out=lha, in_=gA.ap()[:, ms].rearrange("(fc p) s -> p fc s", p=128))
                    nc.sync.dma_start(out=lhb, in_=gB.ap()[:, ms].rearrange("(fc p) s -> p fc s", p=128))
                    pyr = psP.tile([128, 512], F32, tag="yr")
                    pyi = psP.tile([128, 512], F32, tag="yi")
                    for fc in range(16):
                        lh = lha[:, fc, :] if fc < 8 else lhb[:, fc - 8, :]
                        nc.tensor.matmul(pyr, lh, wpr_sb[:, fc, :],
                                         start=(fc == 0), stop=(fc == 15))
                        nc.tensor.matmul(pyi, lh, wpi_sb[:, fc, :],
                                         start=(fc == 0), stop=(fc == 15))
                    syr = ypool.tile([128, 512], F32, tag="syr")
                    nc.scalar.copy(syr, pyr)
                    nc.sync.dma_start(out=y[0, ms, :], in_=syr)
                    syi = ypool.tile([128, 512], F32, tag="syi")
                    nc.vector.tensor_copy(syi, pyi)
                    nc.sync.dma_start(out=y[1, ms, :], in_=syi)

    nc.compile()
    return nc


def _prep_inputs(x_re, x_im, wqkv_re, wqkv_im, wproj_re, wproj_im):
    """Pack per-core input maps (all host-side numpy)."""
    f32 = np.float32

    def qkv_rows(kind, g):
        off = {"q": 0, "k": 1024, "v": 2048}[kind]
        return slice(off + g * 64, off + (g + 1) * 64)

    # mask: cols 0-511 = NEG; cols 512+jj: 0 if jj >= k else NEG
    maskf = np.full((128, 640), NEG, f32)
    k_idx = np.arange(128)[:, None]
    jj = np.arange(128)[None, :]
    maskf[:, 512:] = np.where(jj >= k_idx, 0.0, NEG)

    WA_hg, WB_hg, Wpr_hg, Wpi_hg = {}, {}, {}, {}
    fc_order = [0, 1, 2, 3, 8, 9, 10, 11, 4, 5, 6, 7, 12, 13, 14, 15]
    for hg in range(2):
        WA = np.empty((D, 4096), f32)
        WB = np.empty((D, 4096), f32)
        for h in range(HPC):
            g = hg * HPC + h
            Wqr = wqkv_re[qkv_rows("q", g)]   # [64, D]
            Wqi = wqkv_im[qkv_rows("q", g)]
            Wkr = wqkv_re[qkv_rows("k", g)]
            Wki = wqkv_im[qkv_rows("k", g)]
            Wvr = wqkv_re[qkv_rows("v", g)]
            Wvi = wqkv_im[qkv_rows("v", g)]
            c = h * 128
            # qriT = [qr; qi]
            WA[:, c:c + 64] = Wqr.T;        WA[:, c + 64:c + 128] = Wqi.T
            WB[:, c:c + 64] = -Wqi.T;       WB[:, c + 64:c + 128] = Wqr.T
            # q2T = [qi; -qr]
            c = 1024 + h * 128
            WA[:, c:c + 64] = Wqi.T;        WA[:, c + 64:c + 128] = -Wqr.T
            WB[:, c:c + 64] = Wqr.T;        WB[:, c + 64:c + 128] = Wqi.T
            # kriT = [kr; ki]
            c = 2048 + h * 128
            WA[:, c:c + 64] = Wkr.T;        WA[:, c + 64:c + 128] = Wki.T
            WB[:, c:c + 64] = -Wki.T;       WB[:, c + 64:c + 128] = Wkr.T
            # v natural = [vr; vi]
            c = 3072 + h * 128
            WA[:, c:c + 64] = Wvr.T;        WA[:, c + 64:c + 128] = Wvi.T
            WB[:, c:c + 64] = -Wvi.T;       WB[:, c + 64:c + 128] = Wvr.T
        WA_hg[hg], WB_hg[hg] = np.ascontiguousarray(WA), np.ascontiguousarray(WB)

        cols = slice(hg * 512, (hg + 1) * 512)
        Wpr = np.empty((2048, 512), f32)
        Wpi = np.empty((2048, 512), f32)
        for fci, g in enumerate(fc_order):
            gs = slice(g * 64, (g + 1) * 64)
            r = fci * 128
            Wpr[r:r + 64] = wproj_re[cols, gs].T
            Wpr[r + 64:r + 128] = -wproj_im[cols, gs].T
            Wpi[r:r + 64] = wproj_im[cols, gs].T
            Wpi[r + 64:r + 128] = wproj_re[cols, gs].T
        Wpr_hg[hg], Wpi_hg[hg] = np.ascontiguousarray(Wpr), np.ascontiguousarray(Wpi)

    xT = {}
    for b in range(B):
        xT[b] = (np.ascontiguousarray(x_re[b].T.astype(f32)),
                 np.ascontiguousarray(x_im[b].T.astype(f32)))

    in_maps = []
    for c in range(N_CORES):
        b, hg = c // 2, c % 2
        in_maps.append({
            "xrT": xT[b][0], "xiT": xT[b][1],
            "WA": WA_hg[hg], "WB": WB_hg[hg],
            "Wpr": Wpr_hg[hg], "Wpi": Wpi_hg[hg],
            "maskf": maskf,
        })
    # convert to the DRAM dtypes the kernel declares (cached per id)
    np_s1, np_ot = _np_of(DT_S1), _np_of(DT_OT)
    conv = {}
    def cv(a, npdt):
        key = (id(a), np.dtype(npdt).name)
        if key not in conv:
            conv[key] = np.ascontiguousarray(a.astype(npdt))
        return conv[key]
    for im in in_maps:
        for k in ("xrT", "xiT", "WA", "WB"):
            im[k] = cv(im[k], np_s1)
        for k in ("Wpr", "Wpi"):
            im[k] = cv(im[k], np_ot)
    return in_maps


def _get_nc():
    if "nc" not in _CACHE:
        _CACHE["nc"] = _build()
    return _CACHE["nc"]


def kernel(x_re, x_im, wqkv_re, wqkv_im, wproj_re, wproj_im, _trace=False):
    nc = _get_nc()
    in_maps = _prep_inputs(np.asarray(x_re, np.float32), np.asarray(x_im, np.float32),
                           np.asarray(wqkv_re, np.float32), np.asarray(wqkv_im, np.float32),
                           np.asarray(wproj_re, np.float32), np.asarray(wproj_im, np.float32))
    res = run_bass_kernel_spmd(nc, in_maps, list(range(N_CORES)), trace=_trace)
    out = np.empty((2, B, S, D), np.float32)
    for c in range(N_CORES):
        b, hg = c // 2, c % 2
        yc = res.results[c]["y"]
        out[0, b, :, hg * 512:(hg + 1) * 512] = yc[0]
        out[1, b, :, hg * 512:(hg + 1) * 512] = yc[1]
    if _trace:
        return out, res
    return out
